# revision 32
# baseline (speedup 1.0000x reference)
"""Multi-head causal self-attention on 8 Trainium2 NeuronCores.

Problem: x [4, 2048, 1024], Wq/Wk/Wv/Wo [1024, 1024] (applied as x @ W.T),
16 heads, dk=64, causal softmax, output [4, 2048, 1024], all fp32.

Sharding: 8 cores = 4 batches x 2 head-groups (8 heads each).
Each core computes QKV projections for its 8 heads, streaming causal
attention, and a partial output projection (Wo row-split). The host adds
the two partial outputs per batch element.

Per-core layouts (chosen so NO on-device transposes are needed):
  xT  [1024, 2048]  = x[b].T          (host-transposed)
  wqT [1024, 512]   = (Wq/8).T cols for this head group (1/sqrt(dk) folded)
  wkT [1024, 512], wvT [1024, 512]
  woT [512, 1024]   = Wo[:, cols].T
  QT/KT on chip as [feat, seq] (head pairs stacked on partitions),
  V as [seq, 8 x (64 feat + ones-col)] bf16: the ones column makes each
  ctx matmul (M=65) also produce the softmax denominator in PSUM row 64,
  so no separate ones-matmul is needed AND the ctx matmuls use the same
  full-array PE configuration as the projections (no weight-config switch
  between them, which otherwise serializes LDWEIGHTS against the
  in-flight matmul's fill).
  scoresT tiles [k=128, q<=512] per head pair are exp'ed on ScalarE into
  bf16; the causal mask is applied with affine_select on the idle GpSimd
  engine. Diagonal tiles narrow scores/exp/mask/ctx to the causally-live
  q-range. 1/l comes from a DVE reciprocal of the PSUM denominator row,
  broadcast across partitions by a stride-0 DMA (no PE involvement).

Projection s-chunks and attention q-blocks are interleaved in program
order so TensorE (projections) and ScalarE (exp) work concurrently.
"""

import ml_dtypes
import numpy as np

import concourse.bass as bass
import concourse.mybir as mybir
import concourse.tile as tile
from concourse.bass_utils import run_bass_kernel_spmd
from concourse.vector_clock import ScopedClock

F32 = mybir.dt.float32
BF16 = mybir.dt.bfloat16
AF = mybir.ActivationFunctionType
ALU = mybir.AluOpType

B, S, D = 4, 2048, 1024
H = 16
DK = 64
N_CORES = 8
HG = 512          # head-group width (8 heads x 64)


# ---------------------------------------------------------------------------
# This walrus accepts at most 1 sem wait per instruction (2 for
# EventSemaphore). Tile emits more in two places; both are fixed up here by
# moving excess waits onto preceding instructions on the same engine.
# ---------------------------------------------------------------------------
def _split_drain_and_barrier(self, tick_clock, wait_clock):
    nc = self.nc
    probe = nc.sync.nop(nofuse=True, hint="tile_drain_waits")
    wait_clock.add_sem_waits(
        probe.ins, ScopedClock({None: tick_clock.global_clock})
    )
    si = probe.ins.sync_info
    waits = list(si.on_wait) if si is not None else []
    if len(waits) > 1:
        probe.ins.sync_info = mybir.SyncInfo(on_wait=[waits[0]], on_update=[])
        for w in waits[1:]:
            n = nc.sync.nop(nofuse=True, hint="tile_drain_waits")
            n.ins.sync_info = mybir.SyncInfo(on_wait=[w], on_update=[])
    nc.sync.drain()
    nc.all_engine_barrier()
    popped = nc._tile_sem_poison_stack.pop()
    assert popped is self._sem_poison
    # clear_and_free_semaphores emits a dma_reset+sem_clear gadget PER
    # COMPACT RANGE of semaphore ids; with ~57 fragmented ids that is ~57
    # all-engine barrier rounds (~12us of postamble). Clearing one
    # contiguous range covering all allocated ids (the gaps are our own
    # free-pool ids, harmlessly reset to 0) costs a single gadget.
    sems = list(self.sems.allocated().values())
    if sems:
        nums = sorted(
            s.num if hasattr(s, "num") else int(s) for s in sems
        )
        full = range(nums[0], nums[-1] + 1)
        nc.gpsimd.dma_reset(full)
        nc.gpsimd.sem_clear(full)
        nc._state.prepend_free_semaphores(nums)
    nc.all_engine_barrier()


tile.TileContext._drain_and_barrier = _split_drain_and_barrier

_wsplit_counter = [0]


def _enforce_wait_limits(m):
    for fn in m.functions:
        for bb in fn.blocks:
            out = []
            changed = False
            for inst in bb.instructions:
                si = inst.sync_info
                cap = 2 if isinstance(inst, mybir.InstEventSemaphore) else 1
                if si is not None and len(si.on_wait) > cap:
                    waits = list(si.on_wait)
                    keep, extra = waits[:cap], waits[cap:]
                    for i in range(0, len(extra), 2):
                        _wsplit_counter[0] += 1
                        out.append(mybir.InstEventSemaphore(
                            name=f"I-wsplit-{_wsplit_counter[0]}",
                            engine=inst.engine,
                            ins=[], outs=[],
                            sync_info=mybir.SyncInfo(
                                on_wait=extra[i:i + 2], on_update=[]),
                        ))
                    inst.sync_info = mybir.SyncInfo(
                        on_wait=keep, on_update=list(si.on_update))
                    changed = True
                out.append(inst)
            if changed:
                bb.instructions = out


def build_nc():
    nc = bass.Bass()

    xT = nc.declare_dram_parameter("xT", [D, S], BF16, isOutput=False)
    wqT = nc.declare_dram_parameter("wqT", [D, HG], BF16, isOutput=False)
    wkT = nc.declare_dram_parameter("wkT", [D, HG], BF16, isOutput=False)
    wvT = nc.declare_dram_parameter("wvT", [D, HG], BF16, isOutput=False)
    woT = nc.declare_dram_parameter("woT", [HG, D], BF16, isOutput=False)
    yout = nc.declare_dram_parameter("y", [S, D], F32, isOutput=True)

    KT8 = D // 128   # contraction tiles for the projections
    NP = 4           # head pairs per core
    NS = S // 128    # seq tiles of 128
    VW = 65          # per-head V width: 64 features + ones column

    from contextlib import ExitStack

    with tile.TileContext(nc) as tc, ExitStack() as ctx:
        ep = ctx.enter_context
        qt_pool = ep(tc.tile_pool(name="qt", bufs=4))
        kt_pool = ep(tc.tile_pool(name="kt", bufs=4))
        v_pool = ep(tc.tile_pool(name="v", bufs=16))
        wo_pool = ep(tc.tile_pool(name="wo", bufs=1))
        wq_pool = ep(tc.tile_pool(name="wq", bufs=1))
        wk_pool = ep(tc.tile_pool(name="wk", bufs=1))
        wv_pool = ep(tc.tile_pool(name="wv", bufs=1))
        xt_pool = ep(tc.tile_pool(name="xt", bufs=2))
        exp_pool = ep(tc.tile_pool(name="exp", bufs=3))
        ctxn_pool = ep(tc.tile_pool(name="ctxn", bufs=12))
        cr_pool = ep(tc.tile_pool(name="cr", bufs=4))
        rcp_pool = ep(tc.tile_pool(name="rcp", bufs=2))
        bcs_pool = ep(tc.tile_pool(name="bcs", bufs=4))
        dram_pool = ep(tc.tile_pool(name="ldram", bufs=4, space="DRAM"))
        ybuf_pool = ep(tc.tile_pool(name="ybuf", bufs=2))
        mm_ps = ep(tc.tile_pool(name="mm_ps", bufs=2, space="PSUM"))
        sc_ps = ep(tc.tile_pool(name="sc_ps", bufs=2, space="PSUM"))
        cx_ps = ep(tc.tile_pool(name="cx_ps", bufs=1, space="PSUM"))

        # Warm-up: the PE clock-gate (HAM) starts at 1.2 GHz and flips to
        # 2.4 GHz only after ~3.4us of sustained matmul activity. Burn the
        # initial DMA-wait on dummy matmuls so the real work runs warm.
        warm_pool = ep(tc.tile_pool(name="warm", bufs=1))
        warm_w = warm_pool.tile([128, 128], BF16, tag="ww")
        warm_x = warm_pool.tile([128, 512], BF16, tag="wx")
        nc.vector.memset(warm_w[:], 0.0)
        nc.vector.memset(warm_x[:], 0.0)
        warm_ps = mm_ps.tile([128, 512], F32, tag="mm", name="warm")
        for _ in range(10):
            nc.tensor.matmul(warm_ps[:], warm_w[:], warm_x[:],
                             start=True, stop=True)

        QT = [qt_pool.tile([128, S], BF16, tag="qt", name=f"QT{p}")
              for p in range(NP)]
        KTt = [kt_pool.tile([128, S], BF16, tag="kt", name=f"KTt{p}")
               for p in range(NP)]
        V = [v_pool.tile([128, 8 * VW], BF16, tag="v", name=f"V{s}")
             for s in range(NS)]
        # DMA order matters for startup latency: the first projection
        # psum-group needs wq + chunk-0 x tiles, so those go first, spread
        # over idle queues; wo is not needed until the first output
        # projection (~80us in) and goes last.
        # one big DMA per weight matrix / x-chunk: the DMA queue-issue
        # overhead (~0.65us per DMA) dominated the startup ramp with 8
        # small DMAs each; a single [128, 8x512] tile lands as one
        # transfer and the matmuls slice it
        wqB = wq_pool.tile([128, KT8 * 512], BF16, tag="wq", name="wqB")
        nc.sync.dma_start(
            wqB[:].rearrange("p (kt c) -> p kt c", kt=KT8),
            wqT[:].rearrange("(kt p) c -> p kt c", p=128))
        xts0 = xt_pool.tile([128, KT8 * 512], BF16, tag="xt", name="xt0")
        nc.gpsimd.dma_start(
            xts0[:].rearrange("p (kt c) -> p kt c", kt=KT8),
            xT[:, 0:512].rearrange("(kt p) c -> p kt c", p=128))

        def emit_xt_dmas(st):
            xts = xt_pool.tile([128, KT8 * 512], BF16, tag="xt",
                               name=f"xt{st}")
            nc.gpsimd.dma_start(
                xts[:].rearrange("p (kt c) -> p kt c", kt=KT8),
                xT[:, st * 512:(st + 1) * 512].rearrange(
                    "(kt p) c -> p kt c", p=128))
            return xts

        def proj_items(st, xts):
            """QKV projection work for chunk st as a flat list of closures,
            one instruction each, so they can be sprinkled between attention
            triples at fine grain."""
            items = []

            def qk_group(ot, w_t, dst, name):
                holder = {}

                def mk_mm(kt):
                    def go():
                        if "ps" not in holder:
                            holder["ps"] = mm_ps.tile(
                                [128, 512], F32, tag="mm", name=name)
                        nc.tensor.matmul(
                            holder["ps"][:],
                            w_t[:, kt * 512 + ot * 128:kt * 512 + (ot + 1) * 128],
                            xts[:, kt * 512:(kt + 1) * 512],
                            start=(kt == 0),
                            stop=(kt == KT8 - 1),
                        )
                    return go

                def copy():
                    nc.vector.tensor_copy(
                        dst[ot][:, st * 512:(st + 1) * 512], holder["ps"][:])

                return [mk_mm(kt) for kt in range(KT8)] + [copy]

            def v_group(sub):
                holder = {}

                def mk_mm(kt):
                    def go():
                        if "ps" not in holder:
                            holder["ps"] = mm_ps.tile(
                                [128, 512], F32, tag="mm", name=f"pv{st}{sub}")
                        nc.tensor.matmul(
                            holder["ps"][:],
                            xts[:, kt * 512 + sub * 128:kt * 512 + (sub + 1) * 128],
                            wvB[:, kt * 512:(kt + 1) * 512],
                            start=(kt == 0),
                            stop=(kt == KT8 - 1),
                        )
                    return go

                def copy():
                    vt = V[st * 4 + sub]
                    v3 = vt[:].rearrange("k (h f) -> k h f", f=VW)
                    nc.vector.tensor_copy(
                        v3[:, :, 0:64],
                        holder["ps"][:].rearrange("k (h f) -> k h f", f=64))
                    nc.gpsimd.memset(v3[:, :, 64:65], 1.0)

                return [mk_mm(kt) for kt in range(KT8)] + [copy]

            for ot in range(NP):
                items.extend(qk_group(ot, wqB, QT, f"pq{st}{ot}"))
                items.extend(qk_group(ot, wkB, KTt, f"pk{st}{ot}"))
            for sub in range(4):
                items.extend(v_group(sub))
            return items

        def norm_part1(cxa, cxb, label):
            """Context rows (and the l row) to SBUF; 1/l via a [128, 8]
            partition-packed DVE reciprocal (the iterative divide is ~6.5
            cyc per free-dim element, so spread it across partitions; the
            repack must hop through DRAM since SBUF partitions are
            physical), then partition-broadcast the 1/l rows with
            stride-0-from-DRAM DMAs. Returns tiles for the deferred
            multiplies."""
            crA = cr_pool.tile([65, 512], F32, tag="crA", name=f"crA{label}")
            crB = cr_pool.tile([65, 512], F32, tag="crB", name=f"crB{label}")
            nc.vector.tensor_copy(crA[:], cxa[0:65, :])
            nc.vector.tensor_copy(crB[:], cxb[0:65, :])
            # the A chain rides the sync queue, the B chain the gpsimd
            # queue, halving the serial DMA-hop latency of the chain
            # direct SBUF->SBUF reshape: src [1,512] and dst [128,4] are
            # both well-formed APs with matching q-linear element order
            lpack = rcp_pool.tile([128, 8], F32, tag="lp", name=f"lp{label}")
            nc.sync.dma_start(lpack[:, 0:4], crA[64:65, :])
            nc.sync.dma_start(lpack[:, 4:8], crB[64:65, :])
            rpk = rcp_pool.tile([128, 8], F32, tag="rp", name=f"rp{label}")
            with nc.allow_low_precision("attention 1/l"):
                nc.vector.reciprocal(rpk[:], lpack[:])
            rd = dram_pool.tile([2, 512], F32, tag="rd", name=f"rd{label}")
            nc.sync.dma_start(
                rd[0].rearrange("(p f) -> p f", p=128), rpk[:, 0:4])
            nc.sync.dma_start(
                rd[1].rearrange("(p f) -> p f", p=128), rpk[:, 4:8])
            bcsA = bcs_pool.tile([64, 512], F32, tag="bcsA", name=f"bA{label}")
            bcsB = bcs_pool.tile([64, 512], F32, tag="bcsB", name=f"bB{label}")
            nc.sync.dma_start(bcsA[:], rd[0:1, :].to_broadcast((64, 512)))
            nc.sync.dma_start(bcsB[:], rd[1:2, :].to_broadcast((64, 512)))
            return crA, crB, bcsA, bcsB

        def norm_part2(crA, crB, bcsA, bcsB, label, sink):
            cn = ctxn_pool.tile([128, 512], BF16, tag="cn", name=f"cn{label}")
            nc.vector.tensor_mul(cn[0:64, :], crA[0:64, :], bcsA[:])
            # head b's rows live on partitions 0:64; only a DMA can move
            # them to partitions 64:128 of the combined cn tile
            tmpB = bcs_pool.tile([64, 512], BF16, tag="tb", name=f"tb{label}")
            nc.vector.tensor_mul(tmpB[:], crB[0:64, :], bcsB[:])
            nc.sync.dma_start(cn[64:128, :], tmpB[:])
            sink.append(cn)

        def attention_block(j, fill, ctxn, pending_norms):
            """Causal attention for q-tile j; normalized per-pair context
            tiles are appended to `ctxn`. `fill` is a list of closures (next
            chunk's projections / output projections) sprinkled into the PE
            stream to cover exp-wait stalls. `pending_norms` carries the
            2-deep deferred normalize multiplies ACROSS blocks so the
            block-end flush never puts a DMA-waiting multiply at the DVE
            queue head (it would delay the PSUM-releasing copies behind
            it and stall the next block's first ctx matmuls)."""
            fill = list(fill)
            n_triples = NP * 4 * (j + 1)
            per_triple = -(-len(fill) // n_triples) if fill else 0

            def emit_fill(n):
                for _ in range(n):
                    if not fill:
                        return
                    if fill[0]() is False:
                        return  # head item's inputs not produced yet
                    fill.pop(0)

            def scores(pair, i):
                # diagonal tiles narrow to the causally-live q range
                p = i - 4 * j
                off = 128 * p if p > 0 else 0
                sc = sc_ps.tile([128, 1024], F32, tag="sc",
                                name=f"sc{j}{pair}{i}")
                qa = QT[pair][0:64, j * 512 + off:(j + 1) * 512]
                qb = QT[pair][64:128, j * 512 + off:(j + 1) * 512]
                ka = KTt[pair][0:64, i * 128:(i + 1) * 128]
                kb = KTt[pair][64:128, i * 128:(i + 1) * 128]
                nc.tensor.matmul(
                    sc[:, off:512], ka, qa,
                    start=True, stop=True, tile_position=(0, 0),
                )
                nc.tensor.matmul(
                    sc[:, 512 + off:1024], kb, qb,
                    start=True, stop=True, tile_position=(64, 0),
                )
                return sc

            ni = 4 * (j + 1)

            def emit_ctx(cxa, cxb, pair, et, i):
                first, last = (i == 0), (i == ni - 1)
                p = i - 4 * j
                off = 128 * p if p > 0 else 0
                va = V[i][:, (2 * pair) * VW:(2 * pair) * VW + VW]
                vb = V[i][:, (2 * pair + 1) * VW:(2 * pair + 1) * VW + VW]
                nc.tensor.matmul(
                    cxa[0:VW, off:512], va, et[:, off:512],
                    start=first, stop=last,
                )
                nc.tensor.matmul(
                    cxb[0:VW, off:512], vb, et[:, 512 + off:1024],
                    start=first, stop=last,
                )

            for pair in range(NP):
                cxa = cx_ps.tile([128, 512], F32, tag="cxa",
                                 name=f"cxa{j}{pair}")
                cxb = cx_ps.tile([128, 512], F32, tag="cxb",
                                 name=f"cxb{j}{pair}")
                sc = scores(pair, 0)
                pending = None
                for i in range(ni):
                    p = i - 4 * j
                    et = exp_pool.tile([128, 1024], BF16, tag="exp",
                                       name=f"et{j}{pair}{i}")
                    if p > 0:
                        off = 128 * p
                        sc3 = sc[:].rearrange(
                            "k (h q) -> k h q", h=2)[:, :, off:512]
                        et3 = et[:].rearrange(
                            "k (h q) -> k h q", h=2)[:, :, off:512]
                        nc.scalar.activation(et3, sc3, AF.Exp)
                        nc.gpsimd.affine_select(
                            out=et3, in_=et3,
                            pattern=[[0, 2], [1, 512 - off]],
                            compare_op=ALU.is_ge,
                            fill=0.0, base=0, channel_multiplier=-1,
                        )
                    else:
                        nc.scalar.activation(et[:], sc[:], AF.Exp)
                        if p == 0:
                            # diagonal block: zero the future positions
                            nc.gpsimd.affine_select(
                                out=et[:], in_=et[:],
                                pattern=[[0, 2], [1, 512]],
                                compare_op=ALU.is_ge,
                                fill=0.0, base=0, channel_multiplier=-1,
                            )
                    if i + 1 < ni:
                        sc = scores(pair, i + 1)
                    if pending is not None:
                        emit_ctx(cxa, cxb, pair, *pending)
                        emit_fill(per_triple)
                    pending = (et, i)
                emit_ctx(cxa, cxb, pair, *pending)
                emit_fill(per_triple)
                # defer the multiplies two pairs so their broadcast DMAs
                # have long completed -- a multiply that waits at the DVE
                # queue head delays the PSUM-freeing copies behind it and
                # stalls the PE
                crA, crB, bcsA, bcsB = norm_part1(cxa, cxb, f"{j}{pair}")
                pending_norms.append((crA, crB, bcsA, bcsB, f"{j}{pair}", ctxn))
                if len(pending_norms) > 2:
                    norm_part2(*pending_norms.pop(0))

            if j == 3:
                for pn in pending_norms:
                    norm_part2(*pn)
                del pending_norms[:]
            # drain any remaining fill (all inputs exist by block end)
            while fill:
                assert fill[0]() is not False
                fill.pop(0)

        def outproj_items(j, ctxn):
            """Output projection for q-tile j as fine-grain fill items."""
            items = []

            def group(s4, oh, holder):
                def mk_mm(pair):
                    def go():
                        if len(ctxn) <= pair:
                            return False  # cn not normalized yet
                        if "ps" not in holder:
                            holder["ps"] = mm_ps.tile(
                                [128, 512], F32, tag="mm", name=f"yp{j}{s4}{oh}")
                        nc.tensor.matmul(
                            holder["ps"][:],
                            ctxn[pair][:, s4 * 128:(s4 + 1) * 128],
                            woB[:, pair * D + oh * 512:pair * D + (oh + 1) * 512],
                            start=(pair == 0),
                            stop=(pair == NP - 1),
                        )
                    return go

                def copy():
                    nc.vector.tensor_copy(
                        holder["yb"][:, oh * 512:(oh + 1) * 512], holder["ps"][:])
                    del holder["ps"]

                return [mk_mm(p) for p in range(NP)] + [copy]

            for s4 in range(4):
                srow = j * 4 + s4
                holder = {}

                def alloc_yb(holder=holder, s4=s4):
                    holder["yb"] = ybuf_pool.tile(
                        [128, D], F32, tag="yb", name=f"yb{j}{s4}")

                items.append(alloc_yb)
                for oh in range(2):
                    items.extend(group(s4, oh, holder))

                def dma_out(holder=holder, srow=srow):
                    nc.sync.dma_start(
                        yout[srow * 128:(srow + 1) * 128, :], holder["yb"][:])

                items.append(dma_out)
            return items

        def outproj_drain(j, ctxn):
            """Block 3's output projection, emitted at the end-of-block
            drain. By then the scores PSUM banks are free, so 6 psum
            accumulation groups (2 sc-tile halves x 2 + 2 mm tiles) can be
            open at once: the pair-0..2 matmuls of 6 groups execute while
            the LAST pair's 1/l DMA chain is still in flight, instead of
            head-blocking behind it."""
            def regions():
                sc1 = sc_ps.tile([128, 1024], F32, tag="sc", name=f"yd{j}a")
                sc2 = sc_ps.tile([128, 1024], F32, tag="sc", name=f"yd{j}b")
                return [sc1[:, 0:512], sc1[:, 512:1024],
                        sc2[:, 0:512], sc2[:, 512:1024],
                        mm_ps.tile([128, 512], F32, tag="mm", name=f"yd{j}c")[:],
                        mm_ps.tile([128, 512], F32, tag="mm", name=f"yd{j}d")[:]]

            def mm(ps, gi, pair):
                s4, oh = gi // 2, gi % 2
                nc.tensor.matmul(
                    ps, ctxn[pair][:, s4 * 128:(s4 + 1) * 128],
                    woB[:, pair * D + oh * 512:pair * D + (oh + 1) * 512],
                    start=(pair == 0), stop=(pair == NP - 1),
                )

            ps = regions()
            for pair in range(NP - 1):          # overlaps the 1/l chain
                for gi in range(6):
                    mm(ps[gi], gi, pair)
            for gi in range(6):
                mm(ps[gi], gi, NP - 1)
            ybs = []
            for s4 in range(3):
                yb = ybuf_pool.tile([128, D], F32, tag="yb", name=f"yd{j}{s4}")
                nc.vector.tensor_copy(yb[:, 0:512], ps[2 * s4])
                nc.vector.tensor_copy(yb[:, 512:1024], ps[2 * s4 + 1])
                srow = j * 4 + s4
                nc.sync.dma_start(
                    yout[srow * 128:(srow + 1) * 128, :], yb[:])
            # last s4 reuses the freed sc-psum (rotation waits the copies
            # above, which are already emitted -- FIFO-safe)
            sc3 = sc_ps.tile([128, 1024], F32, tag="sc", name=f"yd{j}e")
            for pair in range(NP):
                mm(sc3[:, 0:512], 6, pair)
                mm(sc3[:, 512:1024], 7, pair)
            yb = ybuf_pool.tile([128, D], F32, tag="yb", name=f"yd{j}3")
            nc.vector.tensor_copy(yb[:, 0:512], sc3[:, 0:512])
            nc.vector.tensor_copy(yb[:, 512:1024], sc3[:, 512:1024])
            nc.sync.dma_start(yout[(j * 4 + 3) * 128:(j * 4 + 4) * 128, :],
                              yb[:])

        # chunk 0 projections run alone; attention block j then carries
        # chunk j+1's projections and block j-1's output projection as PE
        # filler for its exp-wait stalls. Block 3 also carries its own
        # output projection (there is no chunk 4 to project).
        wkB = wk_pool.tile([128, KT8 * 512], BF16, tag="wk", name="wkB")
        nc.scalar.dma_start(
            wkB[:].rearrange("p (kt c) -> p kt c", kt=KT8),
            wkT[:].rearrange("(kt p) c -> p kt c", p=128))
        wvB = wv_pool.tile([128, KT8 * 512], BF16, tag="wv", name="wvB")
        nc.scalar.dma_start(
            wvB[:].rearrange("p (kt c) -> p kt c", kt=KT8),
            wvT[:].rearrange("(kt p) c -> p kt c", p=128))
        for item in proj_items(0, xts0):
            item()
        woB = wo_pool.tile([128, NP * D], BF16, tag="wo", name="woB")
        nc.sync.dma_start(
            woB[:].rearrange("p (c o) -> p c o", c=NP),
            woT[:].rearrange("(c p) o -> p c o", p=128))
        # Output-projection placement: blocks 0-2 are PE-bound (attention +
        # chunk-projection fill exceeds the exp pace) while block 3 -- whose
        # own output projection runs at the drain -- has exp-paced PE idle.
        # outproj(0) fills block 1; outproj(1) AND outproj(2) both fill
        # block 3's idle instead of adding to PE-bound block 2.
        outs = {}
        pending_norms = []
        for j in range(4):
            ctxn = []
            fill = []
            if j + 1 < 4:
                xts = emit_xt_dmas(j + 1)
                fill += proj_items(j + 1, xts)
            if j == 1:
                fill += outs.pop(0)
            elif j == 3:
                fill += outs.pop(1) + outs.pop(2)
            attention_block(j, fill, ctxn, pending_norms)
            if j == 3:
                outproj_drain(3, ctxn)
            else:
                outs[j] = outproj_items(j, ctxn)

    _enforce_wait_limits(nc.m)
    return nc


_NC = None


def _get_nc():
    global _NC
    if _NC is None:
        _NC = build_nc()
    return _NC


def run(x, Wq, Wk, Wv, Wo, trace=False, trace_kwargs=None):
    """Returns (y, BassKernelResults)."""
    x = np.asarray(x, np.float32)
    scale = 1.0 / np.sqrt(DK)
    in_maps = []
    for core in range(N_CORES):
        b, g = core // 2, core % 2
        cols = slice(g * HG, (g + 1) * HG)
        bf = ml_dtypes.bfloat16
        in_maps.append({
            "xT": np.ascontiguousarray(x[b].T).astype(bf),
            "wqT": np.ascontiguousarray(
                np.asarray(Wq, np.float32).T[:, cols] * scale).astype(bf),
            "wkT": np.ascontiguousarray(
                np.asarray(Wk, np.float32).T[:, cols]).astype(bf),
            "wvT": np.ascontiguousarray(
                np.asarray(Wv, np.float32).T[:, cols]).astype(bf),
            "woT": np.ascontiguousarray(
                np.asarray(Wo, np.float32).T[cols, :]).astype(bf),
        })
    kw = dict(trace_kwargs or {})
    res = run_bass_kernel_spmd(
        _get_nc(), in_maps, list(range(N_CORES)), trace=trace, **kw
    )
    y = np.empty((B, S, D), np.float32)
    for b in range(B):
        y[b] = res.results[2 * b]["y"] + res.results[2 * b + 1]["y"]
    return y, res


def kernel(x, Wq, Wk, Wv, Wo):
    y, _ = run(x, Wq, Wk, Wv, Wo)
    return y
